# revision 1
# baseline (speedup 1.0000x reference)
# Trainium2 Bass kernel: single-head causal attention (k.q^T scores, no scale)
# B=16, T=4096, D=64. Data-parallel over batch: 2 batches per NeuronCore x 8.
import numpy as np

B, T, D = 16, 4096, 64
NCORES = 8
BPC = B // NCORES      # batches per core
TT = 512               # t-tile width (one PSUM bank of fp32)
NTT = T // TT          # 8 t tiles
SB = 128               # s block
NSB = T // SB          # 32 s blocks

_cache = {}


def _build():
    from contextlib import ExitStack
    import concourse.bass as bass
    import concourse.mybir as mybir
    import concourse.tile as tile

    f32 = mybir.dt.float32
    f32r = mybir.dt.float32r
    EXP = mybir.ActivationFunctionType.Exp

    nc = bass.Bass("TRN2", target_bir_lowering=False, debug=False,
                   enable_asserts=False)

    xT_d = nc.dram_tensor("xt", [BPC, D, T], f32r, kind="ExternalInput").ap()
    wq_d = nc.dram_tensor("wqt2", [D, 128], f32r, kind="ExternalInput").ap()
    wk_d = nc.dram_tensor("wkt2", [D, 128], f32r, kind="ExternalInput").ap()
    wv_d = nc.dram_tensor("wvt", [D, D], f32r, kind="ExternalInput").ap()
    mk_d = nc.dram_tensor("maskw", [128, 512], f32r, kind="ExternalInput").ap()
    id_d = nc.dram_tensor("ident", [128, 128], f32, kind="ExternalInput").ap()
    on_d = nc.dram_tensor("ones32", [128, 32], f32r, kind="ExternalInput").ap()
    out_d = nc.dram_tensor("out", [BPC, T, D], f32, kind="ExternalOutput").ap()

    with ExitStack() as ctx:
        tc = ctx.enter_context(tile.TileContext(nc))
        consts = ctx.enter_context(tc.tile_pool(name="consts", bufs=1))
        bigp = ctx.enter_context(tc.tile_pool(name="big", bufs=2))
        ptp = ctx.enter_context(tc.tile_pool(name="pt", bufs=3))
        stg = ctx.enter_context(tc.tile_pool(name="stg", bufs=4))
        # PSUM budget: st [128,1024]x2 = 4 banks, tr [128,65]x2 = 2,
        # out [65,512]x2 = 2  -> 8 banks
        pst = ctx.enter_context(tc.tile_pool(name="pst", bufs=2, space="PSUM"))
        pso = ctx.enter_context(tc.tile_pool(name="pso", bufs=2, space="PSUM"))

        wq_sb = consts.tile([D, 128], f32r, tag="wq")
        wk_sb = consts.tile([D, 128], f32r, tag="wk")
        wv_sb = consts.tile([D, D], f32r, tag="wv")
        mk_sb = consts.tile([128, 512], f32r, tag="mk")
        id_sb = consts.tile([128, 128], f32, tag="id")
        # all input loads on SWDGE queue 0 -> single DMA semaphore, so each
        # fp32r matmul (1 ISA wait slot in its LDWEIGHTS) has <=1 wait
        nc.gpsimd.dma_start(wq_sb[:], wq_d[:])
        nc.gpsimd.dma_start(wk_sb[:], wk_d[:])
        nc.gpsimd.dma_start(wv_sb[:], wv_d[:])
        nc.gpsimd.dma_start(mk_sb[:], mk_d[:])
        nc.gpsimd.dma_start(id_sb[:], id_d[:])

        for b in range(BPC):
            # ---- load x^T and project q,k (duplicated on partition halves), v
            xt_sb = bigp.tile([D, T], f32r, tag="xt")
            nc.gpsimd.dma_start(xt_sb[:], xT_d[b])
            qTd = bigp.tile([128, T], f32r, tag="qtd")
            kTd = bigp.tile([128, T], f32r, tag="ktd")
            vex = bigp.tile([128, NSB, 65], f32r, tag="vex")
            nc.gpsimd.dma_start(vex[:, :, 64], on_d[:])

            for i in range(NTT):
                ps = pst.tile([128, 1024], f32, tag="st")
                sl = slice(i * TT, (i + 1) * TT)
                nc.tensor.matmul(ps[:, 0:512], wq_sb[:], xt_sb[:, sl])
                nc.tensor.matmul(ps[:, 512:1024], wk_sb[:], xt_sb[:, sl])
                nc.vector.tensor_copy(qTd[:, sl], ps[:, 0:512])
                nc.vector.tensor_copy(kTd[:, sl], ps[:, 512:1024])

            for c in range(NSB // 2):
                ps = pst.tile([128, 1024], f32, tag="st")
                for p in range(2):
                    tb = 2 * c + p
                    nc.tensor.matmul(
                        ps[:, 512 * p: 512 * p + 64],
                        xt_sb[:, tb * SB:(tb + 1) * SB],
                        wv_sb[:])
                    nc.vector.tensor_copy(vex[:, tb, 0:64],
                                          ps[:, 512 * p: 512 * p + 64])

            # ---- attention: for each t tile, stream s blocks (causal)
            for t in range(NTT):
                outp = pso.tile([65, TT], f32, tag="o")
                n_chunk = 2 * (t + 1)
                for c in range(n_chunk):
                    st = pst.tile([128, 1024], f32, tag="st")
                    pt = ptp.tile([128, 1024], f32r, tag="pt")
                    # two row-tiled score matmuls (even s-block on array rows
                    # 0-63, odd on 64-127 via the duplicated q/k halves)
                    for p in range(2):
                        sblk = 2 * c + p
                        half = slice(64 * p, 64 * (p + 1))
                        nc.tensor.matmul(
                            st[:, 512 * p: 512 * (p + 1)],
                            qTd[half, sblk * SB:(sblk + 1) * SB],
                            kTd[half, t * TT:(t + 1) * TT])
                    nc.scalar.activation(pt[:], st[:], EXP)
                    for p in range(2):
                        sblk = 2 * c + p
                        j = sblk - 4 * t
                        lo = 128 * j if j >= 0 else 0
                        if j >= 0:
                            # diagonal: mask the whole region PV will read, so
                            # PV's read deps only on this DVE write (1 wait)
                            nc.vector.tensor_mul(
                                pt[:, 512 * p + lo: 512 * (p + 1)],
                                pt[:, 512 * p + lo: 512 * (p + 1)],
                                mk_sb[:, 0: TT - lo])
                        nc.tensor.matmul(
                            outp[:, lo:TT],
                            vex[:, sblk, :],
                            pt[:, 512 * p + lo: 512 * (p + 1)],
                            start=(sblk == 0), stop=(sblk == 4 * t + 3))

                # ---- drain: transpose [65,512] -> 4x[128,65], normalize, store
                ot = stg.tile([65, TT], f32, tag="ot")
                nc.vector.tensor_copy(ot[:], outp[:])
                for i in range(4):
                    tr = pst.tile([128, 65], f32, tag="tr")
                    nc.tensor.transpose(tr[:], ot[:, 128 * i: 128 * (i + 1)],
                                        id_sb[0:65, 0:65])
                    rcp = stg.tile([128, 1], f32, tag="rcp")
                    nc.vector.reciprocal(rcp[:], tr[:, 64:65])
                    on = stg.tile([128, 64], f32, tag="on")
                    nc.vector.tensor_scalar_mul(on[:], tr[:, 0:64], rcp[:])
                    r0 = t * TT + 128 * i
                    nc.sync.dma_start(out_d[b, r0:r0 + 128, :], on[:])

    _split_matmul_waits(nc)
    return nc


def _split_matmul_waits(nc):
    """fp32/fp32r matmuls lower via an LDWEIGHTS struct with a single ISA
    wait slot; walrus refuses Matmult instructions carrying >1 sync wait.
    Move every multi-wait Matmult's waits onto a PE NoOp inserted right
    before it (engines execute their stream in order, so this is
    equivalent)."""
    import bass_rust
    import concourse.mybir as mybir
    moved = 0
    for fn in nc.m.functions:
        for bb in fn.blocks:
            il = bb.instructions
            k = 0
            while k < len(il):
                inst = il[k]
                if inst.opcode != "NoOp":
                    si = inst.sync_info
                    if si is not None and si.on_wait and len(si.on_wait) > 1:
                        waits = list(si.on_wait)
                        ups = list(si.on_update) if si.on_update else []
                        # every TPB instruction has a single ISA wait slot:
                        # one NoOp per wait, in order, before the matmul
                        for wi, w in enumerate(waits):
                            nop = mybir.InstNoOp(name=f"{inst.name}-ws{wi}",
                                                 ins=[], outs=[])
                            nop.engine = inst.engine
                            nop.sync_info = bass_rust.SyncInfo(
                                on_wait=[w], on_update=[])
                            il.insert(k, nop)
                            k += 1
                        inst.sync_info = bass_rust.SyncInfo(
                            on_wait=[], on_update=ups)
                        moved += 1
                k += 1
    return moved


def _get_nc():
    if "nc" not in _cache:
        _cache["nc"] = _build()
    return _cache["nc"]


def kernel(x, Wk, Wq, Wv):
    from concourse.bass_utils import run_bass_kernel_spmd

    x = np.ascontiguousarray(np.asarray(x, dtype=np.float32))
    Wk = np.asarray(Wk, dtype=np.float32)
    Wq = np.asarray(Wq, dtype=np.float32)
    Wv = np.asarray(Wv, dtype=np.float32)

    xT = np.ascontiguousarray(x.transpose(0, 2, 1))          # [B, D, T]
    wq2 = np.ascontiguousarray(np.concatenate([Wq.T, Wq.T], axis=1))  # [64,128]
    wk2 = np.ascontiguousarray(np.concatenate([Wk.T, Wk.T], axis=1))
    wvt = np.ascontiguousarray(Wv.T)
    maskw = np.ones((128, 512), dtype=np.float32)
    maskw[:, 0:128] = np.triu(np.ones((128, 128), dtype=np.float32))
    ident = np.eye(128, dtype=np.float32)

    nc = _get_nc()
    in_maps = []
    for c in range(NCORES):
        in_maps.append({
            "xt": np.ascontiguousarray(xT[BPC * c: BPC * (c + 1)]),
            "wqt2": wq2, "wkt2": wk2, "wvt": wvt,
            "maskw": maskw, "ident": ident,
            "ones32": np.ones((128, 32), dtype=np.float32),
        })
    import os
    kw = {}
    if os.environ.get("BASS_TRACE"):
        kw = dict(trace=True, stitch_traces=False)
    res = run_bass_kernel_spmd(nc, in_maps, core_ids=list(range(NCORES)), **kw)
    _cache["last_result"] = res
    out = np.empty((B, T, D), dtype=np.float32)
    for c in range(NCORES):
        out[BPC * c: BPC * (c + 1)] = res.results[c]["out"]
    return out



# revision 2
# speedup vs baseline: 1.3821x; 1.3821x over previous
# Trainium2 Bass kernel: single-head causal attention (k.q^T scores, no scale)
# B=16, T=4096, D=64. Data-parallel over batch: 2 batches per NeuronCore x 8.
# bf16 matmul datapath (fp32 PE mode measures ~3 cyc/row vs bf16 1 cyc/row),
# software-pipelined score->exp->PV chunk loop so PE never waits on Act.
import numpy as np

B, T, D = 16, 4096, 64
NCORES = 8
BPC = B // NCORES      # batches per core
TT = 512               # t-tile width (one PSUM bank of fp32 for PV out)
NTT = T // TT          # 8 t tiles
SB = 128               # s block
NSB = T // SB          # 32 s blocks

_cache = {}


def _build():
    from contextlib import ExitStack
    import concourse.bass as bass
    import concourse.mybir as mybir
    import concourse.tile as tile

    f32 = mybir.dt.float32
    bf16 = mybir.dt.bfloat16
    EXP = mybir.ActivationFunctionType.Exp

    nc = bass.Bass("TRN2", target_bir_lowering=False, debug=False,
                   enable_asserts=False)

    xT_d = nc.dram_tensor("xtb", [BPC, D, T], bf16, kind="ExternalInput").ap()
    wq_d = nc.dram_tensor("wqb", [D, 128], bf16, kind="ExternalInput").ap()
    wk_d = nc.dram_tensor("wkb", [D, 128], bf16, kind="ExternalInput").ap()
    wv_d = nc.dram_tensor("wvb", [D, D], bf16, kind="ExternalInput").ap()
    mk_d = nc.dram_tensor("mkb", [128, 512], bf16, kind="ExternalInput").ap()
    id_d = nc.dram_tensor("identb", [128, 128], f32, kind="ExternalInput").ap()
    on_d = nc.dram_tensor("onesb", [128, 32], bf16, kind="ExternalInput").ap()
    out_d = nc.dram_tensor("out", [BPC, T, D], f32, kind="ExternalOutput").ap()

    with ExitStack() as ctx:
        tc = ctx.enter_context(tile.TileContext(nc))
        consts = ctx.enter_context(tc.tile_pool(name="consts", bufs=1))
        bigp = ctx.enter_context(tc.tile_pool(name="big", bufs=2))
        ptp = ctx.enter_context(tc.tile_pool(name="pt", bufs=3))
        stg = ctx.enter_context(tc.tile_pool(name="stg", bufs=4))
        # PSUM: st [128,1024]f32 x2 = 4 banks, outp [65,512]f32 x2 = 2,
        # tr [128,65]f32 x2 = 2  -> 8 banks
        pst = ctx.enter_context(tc.tile_pool(name="pst", bufs=2, space="PSUM"))
        pso = ctx.enter_context(tc.tile_pool(name="pso", bufs=2, space="PSUM"))
        ptr = ctx.enter_context(tc.tile_pool(name="ptr", bufs=2, space="PSUM"))

        wq_sb = consts.tile([D, 128], bf16, tag="wq")
        wk_sb = consts.tile([D, 128], bf16, tag="wk")
        wv_sb = consts.tile([D, D], bf16, tag="wv")
        mk_sb = consts.tile([128, 512], bf16, tag="mk")
        id_sb = consts.tile([128, 128], f32, tag="id")
        # all input loads on SWDGE queue 0 -> single DMA semaphore, so each
        # matmul (1 ISA wait slot in its LDWEIGHTS) has <=1 wait
        nc.gpsimd.dma_start(wq_sb[:], wq_d[:])
        nc.gpsimd.dma_start(wk_sb[:], wk_d[:])
        nc.gpsimd.dma_start(wv_sb[:], wv_d[:])
        nc.gpsimd.dma_start(mk_sb[:], mk_d[:])
        nc.gpsimd.dma_start(id_sb[:], id_d[:])

        for b in range(BPC):
            # ---- load x^T and project q,k (duplicated on partition halves), v
            xt_sb = bigp.tile([D, T], bf16, tag="xt")
            nc.gpsimd.dma_start(xt_sb[:], xT_d[b])
            qTd = bigp.tile([128, T], bf16, tag="qtd")
            kTd = bigp.tile([128, T], bf16, tag="ktd")
            vex = bigp.tile([128, NSB, 65], bf16, tag="vex")
            nc.gpsimd.dma_start(vex[:, :, 64], on_d[:])

            for i in range(NTT):
                ps = pst.tile([128, 1024], f32, tag="st")
                sl = slice(i * TT, (i + 1) * TT)
                nc.tensor.matmul(ps[:, 0:512], wq_sb[:], xt_sb[:, sl])
                nc.tensor.matmul(ps[:, 512:1024], wk_sb[:], xt_sb[:, sl])
                nc.vector.tensor_copy(qTd[:, sl], ps[:, 0:512])
                nc.vector.tensor_copy(kTd[:, sl], ps[:, 512:1024])

            for c in range(NSB // 2):
                ps = pst.tile([128, 1024], f32, tag="st")
                for p in range(2):
                    tb = 2 * c + p
                    nc.tensor.matmul(
                        ps[:, 512 * p: 512 * p + 64],
                        xt_sb[:, tb * SB:(tb + 1) * SB],
                        wv_sb[:])
                    nc.vector.tensor_copy(vex[:, tb, 0:64],
                                          ps[:, 512 * p: 512 * p + 64])

            # ---- attention: flat software-pipelined chunk stream
            work = [(t, c) for t in range(NTT) for c in range(2 * (t + 1))]

            def scores(t, c):
                st = pst.tile([128, 1024], f32, tag="st")
                for p in range(2):
                    sblk = 2 * c + p
                    half = slice(64 * p, 64 * (p + 1))
                    nc.tensor.matmul(
                        st[:, 512 * p: 512 * (p + 1)],
                        qTd[half, sblk * SB:(sblk + 1) * SB],
                        kTd[half, t * TT:(t + 1) * TT])
                return st

            def mask_pv(t, c, pt, outp):
                for p in range(2):
                    sblk = 2 * c + p
                    j = sblk - 4 * t
                    lo = 128 * j if j >= 0 else 0
                    if j >= 0:
                        # diagonal: mask the whole region PV will read, so
                        # PV's read deps only on this DVE write (1 wait)
                        nc.vector.tensor_mul(
                            pt[:, 512 * p + lo: 512 * (p + 1)],
                            pt[:, 512 * p + lo: 512 * (p + 1)],
                            mk_sb[:, 0: TT - lo])
                    nc.tensor.matmul(
                        outp[:, lo:TT],
                        vex[:, sblk, :],
                        pt[:, 512 * p + lo: 512 * (p + 1)],
                        start=(sblk == 0), stop=(sblk == 4 * t + 3))

            def make_drain(t, outp, b):
                def drain():
                    # transpose [65,512] -> 4x[128,65], normalize, store
                    ot = stg.tile([65, TT], f32, tag="ot")
                    nc.vector.tensor_copy(ot[:], outp[:])
                    for i in range(4):
                        tr = ptr.tile([128, 65], f32, tag="tr")
                        nc.tensor.transpose(tr[:],
                                            ot[:, 128 * i: 128 * (i + 1)],
                                            id_sb[0:65, 0:65])
                        rcp = stg.tile([128, 1], f32, tag="rcp")
                        nc.vector.reciprocal(rcp[:], tr[:, 64:65])
                        on = stg.tile([128, 64], f32, tag="on")
                        nc.vector.tensor_scalar_mul(on[:], tr[:, 0:64], rcp[:])
                        r0 = t * TT + 128 * i
                        nc.sync.dma_start(out_d[b, r0:r0 + 128, :], on[:])
                return drain

            st_cur = scores(*work[0])
            outp = None
            pending_drain = None
            for i, (t, c) in enumerate(work):
                pt = ptp.tile([128, 1024], bf16, tag="pt")
                nc.scalar.activation(pt[:], st_cur[:], EXP)
                if i + 1 < len(work):
                    st_cur = scores(*work[i + 1])
                # emit previous t-tile's drain after the lookahead scores so
                # the PE transposes sit behind them in PE program order
                if pending_drain is not None:
                    pending_drain()
                    pending_drain = None
                if c == 0:
                    outp = pso.tile([65, TT], f32, tag="o")
                mask_pv(t, c, pt, outp)
                if c == 2 * (t + 1) - 1:
                    pending_drain = make_drain(t, outp, b)
            pending_drain()

    _split_matmul_waits(nc)
    return nc


def _split_matmul_waits(nc):
    """Matmults lower via an LDWEIGHTS struct with a single ISA wait slot;
    walrus refuses Matmult instructions carrying >1 sync wait. Move every
    multi-wait Matmult's waits onto a PE NoOp inserted right before it
    (engines execute their stream in order, so this is equivalent)."""
    import bass_rust
    import concourse.mybir as mybir
    moved = 0
    for fn in nc.m.functions:
        for bb in fn.blocks:
            il = bb.instructions
            k = 0
            while k < len(il):
                inst = il[k]
                if inst.opcode != "NoOp":
                    si = inst.sync_info
                    if si is not None and si.on_wait and len(si.on_wait) > 1:
                        waits = list(si.on_wait)
                        ups = list(si.on_update) if si.on_update else []
                        # every TPB instruction has a single ISA wait slot:
                        # one NoOp per wait, in order, before the matmul
                        for wi, w in enumerate(waits):
                            nop = mybir.InstNoOp(name=f"{inst.name}-ws{wi}",
                                                 ins=[], outs=[])
                            nop.engine = inst.engine
                            nop.sync_info = bass_rust.SyncInfo(
                                on_wait=[w], on_update=[])
                            il.insert(k, nop)
                            k += 1
                        inst.sync_info = bass_rust.SyncInfo(
                            on_wait=[], on_update=ups)
                        moved += 1
                k += 1
    return moved


def _get_nc():
    if "nc" not in _cache:
        _cache["nc"] = _build()
    return _cache["nc"]


def kernel(x, Wk, Wq, Wv):
    from concourse.bass_utils import run_bass_kernel_spmd
    import ml_dtypes

    bf = ml_dtypes.bfloat16
    x = np.asarray(x, dtype=np.float32)
    Wk = np.asarray(Wk, dtype=np.float32)
    Wq = np.asarray(Wq, dtype=np.float32)
    Wv = np.asarray(Wv, dtype=np.float32)

    xT = np.ascontiguousarray(x.transpose(0, 2, 1).astype(bf))  # [B, D, T]
    wq2 = np.ascontiguousarray(
        np.concatenate([Wq.T, Wq.T], axis=1).astype(bf))        # [64, 128]
    wk2 = np.ascontiguousarray(
        np.concatenate([Wk.T, Wk.T], axis=1).astype(bf))
    wvt = np.ascontiguousarray(Wv.T.astype(bf))
    maskw = np.ones((128, 512), dtype=bf)
    maskw[:, 0:128] = np.triu(np.ones((128, 128), dtype=np.float32)).astype(bf)
    ident = np.eye(128, dtype=np.float32)

    nc = _get_nc()
    in_maps = []
    for c in range(NCORES):
        in_maps.append({
            "xtb": np.ascontiguousarray(xT[BPC * c: BPC * (c + 1)]),
            "wqb": wq2, "wkb": wk2, "wvb": wvt,
            "mkb": maskw, "identb": ident,
            "onesb": np.ones((128, 32), dtype=bf),
        })
    import os
    kw = {}
    if os.environ.get("BASS_TRACE"):
        kw = dict(trace=True, stitch_traces=False)
    res = run_bass_kernel_spmd(nc, in_maps, core_ids=list(range(NCORES)), **kw)
    _cache["last_result"] = res
    out = np.empty((B, T, D), dtype=np.float32)
    for c in range(NCORES):
        out[BPC * c: BPC * (c + 1)] = res.results[c]["out"]
    return out
